# revision 25
# baseline (speedup 1.0000x reference)
"""Trainium2 Bass kernel for nn_ActQuantWrapper (hadamard + per-token act quant + linear).

Math (per reference):
  z = (H_64 kron I_had) x / 8               -- FHT over 64 groups along feature dim
  sx[t] = clip(absmax(z[t,:])/127, 1e-5)    -- per-token scale
  xq = round(z/sx)*sx                        -- act quant-dequant
  out = xq @ weight.T + bias                 -- weight already per-channel quantized

Device strategy (8 cores, data-parallel over tokens, weight replicated):
  - qx = round(z/sx) is integer in [-127,127]: exactly representable in bf16,
    so the matmul runs at full bf16 PE rate; the result is scaled by sx[t]
    afterward.
  - The weight arrives already per-channel quantized; casting it straight to
    bf16 perturbs each element by <= 2^-9 relative, far inside tolerance, so
    no on-device weight processing is needed.  The host pre-transposes the
    weight into the exact k-major SBUF tile layout so each weight chunk is a
    single fully-linear DMA.
  - The FHT runs in fp16 on the DVE (2-byte dtypes get the 2x element rate);
    the host pre-casts x to fp16.  End-to-end error stays ~4e-3.
  - Rounding uses an fp16 magic-number trick: (z*rx8 + 1536) stored to fp16
    has ulp 1.0, so the store itself rounds to nearest-even integer; a
    subtract then recovers round(z/sx) exactly (integers are exact in bf16).
  - qx is transposed k-major via DMA xbar transpose.
  - A burst of dummy warm-up matmuls keeps the PE p-state high before the
    first real chain arrives.
"""

import numpy as np
import ml_dtypes

import concourse.bass as bass
import concourse.tile as tile
from concourse import bacc, mybir
from concourse.bass_utils import run_bass_kernel_spmd

F32 = mybir.dt.float32
F16 = mybir.dt.float16
BF16 = mybir.dt.bfloat16
MAGIC16 = 1536.0  # 1.5 * 2**10: fp16 ulp is 1.0 here, so the fp16 store rounds

N_CORES = 8
B, S, D_IN, D_OUT = 2, 2048, 4096, 4096
N_TOK = B * S
T_CORE = N_TOK // N_CORES  # 512 tokens per core
N_GROUPS = 64              # hadamard dimension (fixed by reference)
OC_SIZE = 512              # output-chunk width (one PSUM bank of f32)
N_WARMUP = 135             # dummy matmuls to hold the PE p-state before real work


def build_kernel(n_tok, K, O, oc_size, trace_sim=False):
    assert n_tok % 128 == 0 and K % 256 == 0 and O % oc_size == 0
    n_tt = n_tok // 128     # token tiles
    n_kt = K // 128         # contraction tiles
    n_oc = O // oc_size     # output chunks
    had_dim = K // N_GROUPS
    chunk = n_kt * oc_size  # free-dim elems per weight chunk per partition

    nc = bacc.Bacc("TRN2", target_bir_lowering=False, debug=False)
    x_d = nc.dram_tensor("x", [n_tok, K], F16, kind="ExternalInput")
    # weight pre-cast to bf16 and pre-laid-out by the host:
    # w[p, oc*chunk + kt*oc_size + o] = weight[oc*oc_size + o, kt*128 + p]
    w_d = nc.dram_tensor("w", [128, n_oc * chunk], BF16, kind="ExternalInput")
    b_d = nc.dram_tensor("b", [O], F16, kind="ExternalInput")
    out_d = nc.dram_tensor("out", [n_tok, O], F32, kind="ExternalOutput")

    with tile.TileContext(nc, trace_sim=trace_sim) as tc:
        with (
            tc.tile_pool(name="xload", bufs=4) as xload,
            tc.tile_pool(name="xwork", bufs=1) as xwork,
            tc.tile_pool(name="qmagp", bufs=1) as qmagp,
            tc.tile_pool(name="qxp", bufs=1) as qxp,
            tc.tile_pool(name="wpool", bufs=3) as wpool,
            tc.tile_pool(name="outp", bufs=3) as outp,
            tc.tile_pool(name="consts", bufs=1) as consts,
            tc.tile_pool(name="psum", bufs=1, space=bass.MemorySpace.PSUM) as psum,
        ):
            qxT = consts.tile([128, n_kt, n_tok], BF16)
            sx_all = consts.tile([128, n_tt], F32)
            m16 = consts.tile([128, n_tt, 3], F16)   # absmax halves + combined
            xsc = consts.tile([128, n_tt, 3], F32)   # m, r, r8 per token tile
            bb = consts.tile([128, O], F16)          # bias broadcast over partitions
            dum = consts.tile([128, oc_size], BF16)  # warm-up matmul operand

            nc.gpsimd.memset(dum[:], 0.0)
            # warm the Act queue: loads the Copy activation table immediately
            # instead of on the latency-critical first transpose/drain
            nc.scalar.activation(
                out=dum[:, 0:1], in_=dum[:, 0:1],
                func=mybir.ActivationFunctionType.Copy, bias=0.0, scale=1.0,
            )
            srcb = b_d.ap()[:]
            nc.gpsimd.dma_start(
                out=bb[:],
                in_=bass.AP(tensor=srcb.tensor, offset=srcb.offset,
                            ap=[[0, 128]] + list(srcb.ap)),
            )
            # All x tiles load first on the sync queue; weight chunks queue
            # BEHIND them (same queue) so the 12.6MB of weight prefetch cannot
            # crowd out the latency-critical first x tile.  Later chunks are
            # issued lazily inside the chain loop right after the chain that
            # frees their pool slot.
            za_tiles = []
            for tt in range(n_tt):
                za = xload.tile([128, K], F16, tag="za", name=f"za{tt}")
                nc.sync.dma_start(za[:], x_d.ap()[tt * 128:(tt + 1) * 128, :])
                za_tiles.append(za)

            n_wbufs = 3
            qw_tiles = [None] * n_oc

            def issue_w(oc):
                qw = wpool.tile([128, n_kt, oc_size], BF16, tag="qw", name=f"qw{oc}")
                nc.sync.dma_start(
                    qw[:], w_d.ap()[:, oc * chunk:(oc + 1) * chunk])
                qw_tiles[oc] = qw
            # NOTE: the first n_wbufs chunks are emitted after x tile 0 below —
            # emission order controls DMA-completion-semaphore pool slots, and
            # emitting them here would make tile 0's transposes alias the slow
            # weight transfers (an observed ~8us stall before the first chain).

            # warm-up: keep the PE busy (and its p-state high) while the
            # x path computes the first token tiles
            for i in range(N_WARMUP):
                pw = psum.tile([128, oc_size], F32, tag=f"ps{i % 8}")
                nc.tensor.matmul(pw[:], dum[:, 0:128], dum[:],
                                 start=True, stop=True)

            # ---------------- matmul chain schedule ----------------
            # Interleave the first min(3, n_oc) chunks across token tiles so the
            # PE has 3 chains of work each time a token tile becomes ready
            # during the x-phase ramp; remaining chunks run sequentially.
            n_il = min(3, n_oc)
            chains = [(oc, t) for t in range(n_tt) for oc in range(n_il)]
            chains += [(oc, t) for oc in range(n_il, n_oc) for t in range(n_tt)]

            last_pos = {}
            for ci, (oc, t) in enumerate(chains):
                last_pos[oc] = ci
            emit_after = {}  # chain idx -> chunks whose pool slot frees there
            for oc in range(n_wbufs, n_oc):
                emit_after.setdefault(last_pos[oc - n_wbufs], []).append(oc)

            def emit_chain(ci):
                oc, t = chains[ci]
                qw = qw_tiles[oc]
                ps = psum.tile([128, oc_size], F32, tag=f"ps{ci % 8}",
                               name=f"ps_{ci}")
                for k in range(n_kt):
                    nc.tensor.matmul(
                        ps[:],
                        qxT[:, k, t * 128:(t + 1) * 128],
                        qw[:, k, :],
                        start=(k == 0), stop=(k == n_kt - 1),
                    )
                o_sb = outp.tile([128, oc_size], F32, tag="osb", name=f"osb_{ci}")
                # out = psum * sx[t] + bias: scale on the Act engine (gpsimd
                # cannot read PSUM), bias added in-place on gpsimd; keeps the
                # Vector queue free for the FHT during the ramp
                nc.scalar.activation(
                    out=o_sb[:], in_=ps[:],
                    func=mybir.ActivationFunctionType.Copy,
                    bias=0.0, scale=sx_all[:, t:t + 1],
                )
                nc.gpsimd.tensor_add(
                    o_sb[:], o_sb[:], bb[:, oc * oc_size:(oc + 1) * oc_size])
                nc.sync.dma_start(
                    out_d.ap()[t * 128:(t + 1) * 128,
                               oc * oc_size:(oc + 1) * oc_size],
                    o_sb[:],
                )
                for j in emit_after.get(ci, ()):
                    issue_w(j)

            # ---------------- x path: fp16 FHT -> quant -> transpose ----------------
            # chains for token tile t-1 are emitted after x tile t so every
            # engine queue interleaves drains between successive tiles' work
            for tt in range(n_tt):
                za = za_tiles[tt]
                zb = xwork.tile([128, K], F16, tag="zb")
                bufs = [za, zb]
                for s in range(6):
                    src, dst = bufs[s % 2], bufs[(s + 1) % 2]
                    blk = had_dim << s
                    sv = src[:].rearrange("p (a c b) -> p a c b", c=2, b=blk)
                    dv = dst[:].rearrange("p (a c b) -> p a c b", c=2, b=blk)
                    nc.vector.tensor_add(dv[:, :, 0, :], sv[:, :, 0, :], sv[:, :, 1, :])
                    nc.vector.tensor_sub(dv[:, :, 1, :], sv[:, :, 0, :], sv[:, :, 1, :])
                # 6 stages end back in za (unscaled by 1/8; folded into the scale)
                KH = K // 2
                m = m16[:, tt, 2:3]
                nc.vector.tensor_reduce(
                    out=m, in_=za[:], axis=mybir.AxisListType.X,
                    op=mybir.AluOpType.max, apply_absolute_value=True,
                )
                # sx = clip((m/8)/127, 1e-5) = clip(m/1016, 1e-5); m/8 is exact
                nc.vector.tensor_scalar(
                    out=sx_all[:, tt:tt + 1], in0=m,
                    scalar1=float(np.float32(1.0) / np.float32(1016.0)),
                    scalar2=1e-5,
                    op0=mybir.AluOpType.mult, op1=mybir.AluOpType.max,
                )
                rx = xsc[:, tt, 1:2]
                nc.vector.reciprocal(rx, sx_all[:, tt:tt + 1])
                rx8 = xsc[:, tt, 2:3]
                nc.vector.tensor_scalar_mul(rx8, rx, 0.125)
                # qmag = fp16(za*rx8 + 1536): the fp16 store rounds to integer.
                # K-halves so the first transpose (and the matmuls reading its
                # k-tiles) can start while the second half is still quantizing.
                qmag = qmagp.tile([128, K], F16, tag="qm")
                qx = qxp.tile([128, K], BF16, tag="qx")
                for h, (lo, hi) in enumerate(((0, KH), (KH, K))):
                    nc.vector.tensor_scalar(
                        out=qmag[:, lo:hi], in0=za[:, lo:hi],
                        scalar1=rx8, scalar2=MAGIC16,
                        op0=mybir.AluOpType.mult, op1=mybir.AluOpType.add,
                    )
                    nc.vector.tensor_scalar_add(
                        qx[:, lo:hi], qmag[:, lo:hi], -MAGIC16)
                    nc.scalar.dma_start_transpose(
                        qxT[:, (lo // 128):(hi // 128), tt * 128:(tt + 1) * 128],
                        qx[:, lo:hi],
                    )
                if tt == 0:
                    for oc in range(min(n_wbufs, n_oc)):
                        issue_w(oc)
                if tt >= 1:
                    for ci in range(n_il * (tt - 1), n_il * tt):
                        emit_chain(ci)

            for ci in range(n_il * (n_tt - 1), len(chains)):
                emit_chain(ci)

    nc.compile()
    return nc


_CACHED = None


def _get_full_kernel():
    global _CACHED
    if _CACHED is None:
        _CACHED = build_kernel(T_CORE, D_IN, D_OUT, OC_SIZE)
    return _CACHED


def _pack_weight(weight):
    """[O, K] f32 -> [128, n_oc*n_kt*oc_size] bf16 in the per-partition tile
    layout the kernel DMAs linearly: w[p, ((oc*n_kt)+kt)*oc_size + o] =
    weight[oc*oc_size + o, kt*128 + p]."""
    n_oc = D_OUT // OC_SIZE
    n_kt = D_IN // 128
    wb = np.asarray(weight, dtype=np.float32).astype(ml_dtypes.bfloat16)
    wb = wb.reshape(n_oc, OC_SIZE, n_kt, 128).transpose(3, 0, 2, 1)
    return np.ascontiguousarray(wb).reshape(128, n_oc * n_kt * OC_SIZE)


def kernel(x, weight, bias, had_dim):
    assert int(had_dim) == 64
    assert x.shape == (B, S, D_IN) and weight.shape == (D_OUT, D_IN)
    nc = _get_full_kernel()
    xf = np.asarray(x).reshape(N_TOK, D_IN).astype(np.float16)
    w = _pack_weight(weight)
    bi = np.asarray(bias).astype(np.float16)
    in_maps = [
        {"x": xf[i * T_CORE:(i + 1) * T_CORE], "w": w, "b": bi}
        for i in range(N_CORES)
    ]
    res = run_bass_kernel_spmd(nc, in_maps, core_ids=list(range(N_CORES)))
    out = np.concatenate([r["out"] for r in res.results], axis=0)
    return out.reshape(B, S, D_OUT)


if __name__ == "__main__":
    rng = np.random.default_rng(0)
    x = rng.standard_normal((B, S, D_IN), dtype=np.float32)
    w = rng.standard_normal((D_OUT, D_IN), dtype=np.float32)
    b = rng.standard_normal(D_OUT).astype(np.float32)
    o = kernel(x, w, b, np.int64(64))
    print(o.shape, o.dtype)
